# revision 16
# baseline (speedup 1.0000x reference)
"""Trainium2 Bass kernel for a 4-layer post-LN transformer encoder.

Sharding: 8 cores = 2 batch groups x 4-way sequence shard (256 tokens/core).
Per layer each core computes QKV for its own tokens, AllGathers K,V within
its 4-core batch group, then runs attention/FFN for its tokens only.
Activations are kept feature-major ([D, T]) on chip so every matmul consumes
natural layouts (weights as lhsT, activations as rhs) with zero transposes.
GEMMs run in bf16 (weights pre-cast on host), accumulation fp32 in PSUM;
softmax and layernorm run in fp32.

v2 over the baseline:
- K and V travel through the AllGathers in fp8e4 (a x16 pre-scale folded
  into Wk/Wv on the host, unfolded through Wq and the softmax-denominator
  ones-columns).  Halves collective time.  The PE consumes fp8 K/V directly
  (fp8 lhsT x bf16 rhs is exact on HW; fp8 x fp8 for ctx).
- The K gather is split into two feature-halves so heads 0-7 can start
  scoring while the second half and V are still on the wire.
- Attention runs all 16 heads' scores+exp before any ctx matmul, so the V
  gather hides entirely behind score/exp work; softmax probabilities are
  staged in fp8 (exp gets a -ln16 bias, cancelled by the 16.0 denominator
  ones-columns, keeping exp outputs well inside fp8 range) which both fits
  16 heads in SBUF and enables a DoubleRow fp8 ctx matmul.
- All collective-path DMAs (bounce-out, gathers, shard loads) sit on the
  gpsimd SWDGE ring so their semaphore waits never head-of-line block the
  weight-streaming sync ring.
- A dummy 256B AllGather at kernel start eats the one-time ~31us comm-init
  barrier under the prologue.
- V is transported in its augmented interleaved layout (ones included) so
  post-gather loads are long contiguous lines.
- Next layer's Wk/Wv stream in before the FFN2 weight stream.
- LayerNorm rstd is a single Rsqrt activation.

Attention per head: scores are computed transposed ([k_tokens, q_tokens]) so
the pad-mask (a per-key bias, i.e. per-partition) and the exp both fold into
the single PSUM-eviction activation. The softmax denominator falls out of the
ctx matmul via a ones-column interleaved into V; division uses a reciprocal
row broadcast across partitions with a tiny K=2 PE outer-product.
"""

import numpy as np
import ml_dtypes

import concourse.bass as bass
import concourse.mybir as mybir
import concourse.tile as tile
from concourse import bacc
from concourse.bass_utils import run_bass_kernel_spmd

# model dims (hardcoded per problem spec)
B, S, D, H, DK, DV, F, L, V = 2, 1024, 1024, 16, 64, 64, 4096, 4, 32000
PAD = 0
EPS = 1e-5
P = 128
NCORES = 8
GROUPS = [[0, 1, 2, 3], [4, 5, 6, 7]]
T = S // 4          # tokens per core
NC = D // P         # feature chunks (8)
FC = F // P         # ffn feature chunks (32)
NSH = 4             # shards per group
TC = T // P         # token chunks per core (2)
SCALE = 1.0 / np.sqrt(DK)
KVS = 16.0          # fp8 transport pre-scale for K and V (power of two)
E = DV + 1          # head stride in augmented V
HE = H * E          # 1040

f32 = mybir.dt.float32
bf16 = mybir.dt.bfloat16
f8e4 = mybir.dt.float8e4

AF = mybir.ActivationFunctionType
ALU = mybir.AluOpType
DR = mybir.MatmulPerfMode.DoubleRow


def posenc_np(seq_len, dim):
    pos = np.arange(seq_len, dtype=np.float32)[:, None]
    div = np.exp(-(np.arange(0, dim, 2, dtype=np.float32) / dim) * np.log(10000.0))
    pe = np.zeros((seq_len, dim), np.float32)
    pe[:, 0::2] = np.sin(pos * div)
    pe[:, 1::2] = np.cos(pos * div)
    return pe


def build_nc():
    nc = bacc.Bacc(None, target_bir_lowering=False, num_devices=NCORES)

    # ---- DRAM parameters (per-core) ----
    x0t = nc.declare_dram_parameter("x0t", [NC, P, T], f32, isOutput=False)
    maskcol = nc.declare_dram_parameter("maskcol", [P, NC], f32, isOutput=False)
    wq = nc.declare_dram_parameter("wq", [L, D, D], bf16, isOutput=False)
    wk = nc.declare_dram_parameter("wk", [L, D, D], bf16, isOutput=False)
    wv = nc.declare_dram_parameter("wv", [L, D, D], bf16, isOutput=False)
    wo = nc.declare_dram_parameter("wo", [L, D, D], bf16, isOutput=False)
    w1 = nc.declare_dram_parameter("w1", [L, D, F], bf16, isOutput=False)
    w2 = nc.declare_dram_parameter("w2", [L, F, D], bf16, isOutput=False)
    out = nc.declare_dram_parameter("out", [NC, P, T], f32, isOutput=True)

    with tile.TileContext(nc) as tc:
        with (
            tc.tile_pool(name="persist", bufs=1) as persist,
            tc.tile_pool(name="wkv", bufs=16) as wkv,
            tc.tile_pool(name="wqo", bufs=10) as wqo,
            tc.tile_pool(name="w1p", bufs=24) as w1p,
            tc.tile_pool(name="w2p", bufs=4) as w2p,
            tc.tile_pool(name="pTp", bufs=16) as pTp,
            tc.tile_pool(name="rows", bufs=3) as rows,
            tc.tile_pool(name="ps_main", bufs=2, space="PSUM") as ps_main,
            tc.tile_pool(name="ps_attn", bufs=2, space="PSUM") as ps_attn,
            tc.tile_pool(name="ps_ctx", bufs=2, space="PSUM") as ps_ctx,
            tc.tile_pool(name="ps_aux", bufs=2, space="PSUM") as ps_aux,
            tc.tile_pool(name="dram", bufs=1, space="DRAM") as dram,
        ):
            # ---- persistent SBUF state ----
            x = persist.tile([P, NC, T], f32, name="x")           # residual stream
            xb = persist.tile([P, NC, T], bf16, name="xb")        # bf16 copy
            z = persist.tile([P, NC, T], f32, name="z")           # residual sum
            x1b = persist.tile([P, NC, T], bf16, name="x1b")      # post-LN1 (bf16)
            zb = persist.tile([P, NC, T], bf16, name="zb")        # LN scratch
            sq = persist.tile([P, NC, T], bf16, name="sq")        # LN scratch
            qT = persist.tile([P, NC, T], bf16, name="qT")
            kTl = persist.tile([P, NC, T], f8e4, name="kTl")      # local K^T staging
            vl = persist.tile([P, TC, HE], f8e4, name="vl")       # local augmented V
            kT = persist.tile([P, NSH, NC, T], f8e4, name="kT")   # gathered K^T
            vaug = persist.tile([P, NSH, TC, HE], f8e4, name="vaug")
            ctx_un = persist.tile([P, NC, T], bf16, name="ctx_un")
            ctxT = persist.tile([P, NC, T], bf16, name="ctxT")
            hT = persist.tile([P, FC, T], bf16, name="hT")
            mask_sb = persist.tile([P, NC], f32, name="mask_sb")
            ones_col = persist.tile([P, 1], bf16, name="ones_col")
            ones_row = persist.tile([1, P], f32, name="ones_row")
            ones_row_bf = persist.tile([1, P], bf16, name="ones_row_bf")
            eps1 = persist.tile([1, 1], f32, name="eps1")
            zcol = persist.tile([P, 1], f32, name="zcol")

            # DRAM bounce buffers for the per-group K and V AllGathers (fp8).
            dummy_in = dram.tile([4, 64], f8e4, name="dummy_in")
            dummy_out = dram.tile([NSH, 4, 64], f8e4, name="dummy_out")
            k_in_a = dram.tile([P, NC // 2, T], f8e4, name="k_in_a")
            k_in_b = dram.tile([P, NC // 2, T], f8e4, name="k_in_b")
            v_in = dram.tile([P, TC, HE], f8e4, name="v_in")
            k_out_a = dram.tile([NSH, P, NC // 2, T], f8e4, name="k_out_a")
            k_out_b = dram.tile([NSH, P, NC // 2, T], f8e4, name="k_out_b")
            v_out = dram.tile([NSH, P, TC, HE], f8e4, name="v_out")

            # ---- prologue ----
            # dummy collective eats the one-time comm-init barrier early
            nc.gpsimd.collective_compute(
                "AllGather", ALU.bypass, replica_groups=GROUPS,
                ins=[dummy_in.opt()], outs=[dummy_out.opt()])
            nc.sync.dma_start(x[:], x0t[:].rearrange("c p t -> p c t"))
            nc.sync.dma_start(mask_sb[:], maskcol[:])
            nc.vector.memset(ones_col[:], 1.0)
            nc.vector.memset(ones_row[:], 1.0)
            nc.vector.memset(ones_row_bf[:], 1.0)
            nc.vector.memset(eps1[:], EPS)
            nc.vector.memset(zcol[:], 0.0)
            # denominator ones-columns carry the same KVS scale as V so the
            # ratio cancels; they ride along the V gather (only the local
            # staging tile needs the preset).
            nc.vector.memset(
                vl[:].rearrange("p t (h e) -> p t h e", e=E)[:, :, :, DV:], KVS)
            nc.vector.tensor_copy(xb[:], x[:])
            # ~7us of throwaway matmuls warm the HAM clock gate while the
            # prologue DMAs land, so layer 0 starts at full PE clock
            ps_w = ps_aux.tile([P, 512], f32, name="ps_w", tag="ax")
            for _ in range(64):
                nc.tensor.matmul(ps_w[0:1, :T], lhsT=ones_col[:],
                                 rhs=zb[:, 0, :], start=True, stop=True)

            def ln_prep_chunk(m):
                """bf16 cast + square for chunk m (DVE/ACT, off the PE path)."""
                nc.scalar.copy(zb[:, m, :], z[:, m, :])
                nc.vector.tensor_mul(sq[:, m, :], zb[:, m, :], zb[:, m, :])

            def ln_stats_chunk(ps_mean, ps_sq, m):
                """Stats matmuls for chunk m (emitted lag-1 so PE never waits)."""
                nc.tensor.matmul(ps_mean[0:1, :T], lhsT=ones_col[:],
                                 rhs=zb[:, m, :], start=(m == 0), stop=(m == NC - 1))
                nc.tensor.matmul(ps_sq[0:1, :T], lhsT=ones_col[:],
                                 rhs=sq[:, m, :], start=(m == 0), stop=(m == NC - 1))

            def ln_tail(ps_mean, ps_sq, z_in, x_out, xb_out):
                """LayerNorm tail: row math + partition-broadcast + per-chunk
                normalize (per-chunk so the next GEMM phase starts early)."""
                m_row = rows.tile([1, T], f32, name="m_row")
                msq = rows.tile([1, T], f32, name="msq")
                var = rows.tile([1, T], f32, name="var")
                rstd = rows.tile([1, T], f32, name="rstd")
                mrs = rows.tile([1, T], f32, name="mrs")
                nc.vector.tensor_scalar_mul(m_row[:], ps_mean[0:1, :T], 1.0 / D)
                nc.vector.tensor_mul(msq[:], m_row[:], m_row[:])
                nc.vector.scalar_tensor_tensor(
                    var[:], in0=ps_sq[0:1, :T], scalar=1.0 / D, in1=msq[:],
                    op0=ALU.mult, op1=ALU.subtract)
                # rstd = exp(-0.5*ln(var+eps)) keeps ACT in the ln/exp table set
                nc.scalar.activation(var[:], var[:], AF.Ln, bias=eps1[:], scale=1.0)
                nc.scalar.activation(rstd[:], var[:], AF.Exp, bias=zcol[0:1, :],
                                     scale=-0.5)
                nc.vector.tensor_mul(mrs[:], m_row[:], rstd[:])
                ps_r = ps_aux.tile([P, 512], f32, name="ps_r", tag="ax")
                ps_m2 = ps_aux.tile([P, 512], f32, name="ps_m2", tag="ax")
                nc.tensor.matmul(ps_r[:, :T], lhsT=ones_row[:], rhs=rstd[:],
                                 start=True, stop=True)
                nc.tensor.matmul(ps_m2[:, :T], lhsT=ones_row[:], rhs=mrs[:],
                                 start=True, stop=True)
                rb = ps_r[:, None, :T].broadcast_to([P, 2, T])
                mb = ps_m2[:, None, :T].broadcast_to([P, 2, T])
                for c in range(0, NC, 2):
                    if x_out is not None:
                        xo = x_out[:, c:c + 2, :]
                        nc.vector.tensor_mul(xo, z_in[:, c:c + 2, :], rb)
                        nc.vector.tensor_sub(xo, xo, mb)
                        nc.scalar.copy(xb_out[:, c:c + 2, :], xo)
                    else:
                        xo = xb_out[:, c:c + 2, :]
                        nc.vector.tensor_mul(xo, z_in[:, c:c + 2, :], rb)
                        nc.vector.tensor_sub(xo, xo, mb)

            def load_w8(pool, src, l, tag, nm):
                t = [pool.tile([P, D], bf16, name=f"{nm}_{l}_{c}", tag=tag)
                     for c in range(NC)]
                for c in range(NC):
                    nc.sync.dma_start(t[c][:], src[l, c * P:(c + 1) * P, :])
                return t

            # layer-0 K/V weights load during the prologue
            wk_sb = load_w8(wkv, wk, 0, "w", "wk")
            wv_sb = load_w8(wkv, wv, 0, "w", "wv")

            # ---- layers ----
            for l in range(L):
                # --- K projection -> fp8 kTl; gather in two feature halves ---
                for m in range(0, NC, 2):
                    ps = ps_main.tile([P, 512], f32, name="ps_k", tag="mm")
                    for half in range(2):
                        for c in range(NC):
                            nc.tensor.matmul(
                                ps[:, half * T:(half + 1) * T],
                                lhsT=wk_sb[c][:, (m + half) * P:(m + half + 1) * P],
                                rhs=xb[:, c, :], start=(c == 0), stop=(c == NC - 1))
                    nc.scalar.copy(
                        kTl[:, m:m + 2, :].rearrange("p a t -> p (a t)"), ps[:])
                    if m == 2:
                        nc.gpsimd.dma_start(k_in_a[:], kTl[:, 0:4, :])
                        nc.gpsimd.collective_compute(
                            "AllGather", ALU.bypass, replica_groups=GROUPS,
                            ins=[k_in_a.opt()], outs=[k_out_a.opt()])
                nc.gpsimd.dma_start(k_in_b[:], kTl[:, 4:8, :])
                nc.gpsimd.collective_compute(
                    "AllGather", ALU.bypass, replica_groups=GROUPS,
                    ins=[k_in_b.opt()], outs=[k_out_b.opt()])

                # --- Q projection (overlaps the K gathers) ---
                wq_sb = load_w8(wqo, wq, l, "w", "wq")
                for m in range(0, NC, 2):
                    ps = ps_main.tile([P, 512], f32, name="ps_q", tag="mm")
                    for half in range(2):
                        for c in range(NC):
                            nc.tensor.matmul(
                                ps[:, half * T:(half + 1) * T],
                                lhsT=wq_sb[c][:, (m + half) * P:(m + half + 1) * P],
                                rhs=xb[:, c, :], start=(c == 0), stop=(c == NC - 1))
                    nc.vector.tensor_copy(
                        qT[:, m:m + 2, :].rearrange("p a t -> p (a t)"), ps[:])
                # Wo streams in behind wq (slots free after Q-proj)
                wo_sb = load_w8(wqo, wo, l, "w", "wo")

                # --- V projection -> fp8 augmented layout; single gather ---
                for t in range(TC):
                    for nh in range(2):
                        ps = ps_main.tile([P, 512], f32, name="ps_v", tag="mm")
                        for c in range(NC):
                            nc.tensor.matmul(
                                ps[:], lhsT=xb[:, c, t * P:(t + 1) * P],
                                rhs=wv_sb[c][:, nh * 512:(nh + 1) * 512],
                                start=(c == 0), stop=(c == NC - 1))
                        nc.scalar.copy(
                            vl[:, t, :].rearrange(
                                "p (h e) -> p h e", e=E)[:, nh * 8:(nh + 1) * 8, :DV],
                            ps[:].rearrange("p (h d) -> p h d", d=DV))
                nc.gpsimd.dma_start(v_in[:], vl[:])
                nc.gpsimd.collective_compute(
                    "AllGather", ALU.bypass, replica_groups=GROUPS,
                    ins=[v_in.opt()], outs=[v_out.opt()])

                # --- pull gathered K/V shards into SBUF (gpsimd ring) ---
                for sh in range(NSH):
                    nc.gpsimd.dma_start(kT[:, sh, 0:4, :], k_out_a[sh])
                for sh in range(NSH):
                    nc.gpsimd.dma_start(kT[:, sh, 4:8, :], k_out_b[sh])
                for sh in range(NSH):
                    nc.gpsimd.dma_start(vaug[:, sh, :, :], v_out[sh])

                # --- attention phase 1: scores + exp for all heads ---
                # (heads 0-7 touch only K-half A; V still on the wire)
                pTs = []
                for h in range(H):
                    po = (h % 2) * DV
                    cc = h // 2
                    pT = pTp.tile([P, NC, T], f8e4, name="pT")
                    pTs.append(pT)
                    for sh in range(NSH):
                        ps_s = ps_attn.tile([P, 512], f32, name="ps_s", tag="sc")
                        for j in range(TC):
                            nc.tensor.matmul(
                                ps_s[:, j * T:(j + 1) * T],
                                lhsT=kT[po:po + DV, sh, cc, j * P:(j + 1) * P],
                                rhs=qT[po:po + DV, cc, :], start=True, stop=True)
                        for j in range(TC):
                            g = sh * TC + j
                            nc.scalar.activation(
                                pT[:, g, :], ps_s[:, j * T:(j + 1) * T],
                                AF.Exp, bias=mask_sb[:, g:g + 1], scale=1.0)

                # --- attention phase 2: ctx (DoubleRow fp8), divide ---
                ps_b = None
                for h in range(H):
                    po = (h % 2) * DV
                    cc = h // 2
                    pT = pTs[h]
                    if h % 2 == 0:
                        ps_c = ps_ctx.tile([P, T], f32, name="ps_c", tag="cx")
                    else:
                        ps_c = ps_main.tile([P, T], f32, name="ps_c2", tag="mm")
                    for sh in range(NSH):
                        nc.tensor.matmul(
                            ps_c[:E, :],
                            lhsT=vaug[:, sh, :, h * E:(h + 1) * E],
                            rhs=pT[:, sh * TC:(sh + 1) * TC, :],
                            start=(sh == 0), stop=(sh == NSH - 1),
                            perf_mode=DR)
                    rp = rows.tile([1, T], f32, name="rp")
                    rp_bf = rows.tile([1, T], bf16, name="rp_bf")
                    nc.vector.reciprocal(rp[:], ps_c[DV:E, :])
                    nc.vector.tensor_copy(rp_bf[:], rp[:])
                    nc.vector.tensor_copy(ctx_un[po:po + DV, cc, :], ps_c[:DV, :])
                    if h % 2 == 0:
                        ps_b = ps_aux.tile([P, 512], f32, name="ps_b", tag="ax")
                    nc.tensor.matmul(ps_b[po:po + DV, :T],
                                     lhsT=ones_row_bf[0:1, :DV], rhs=rp_bf[:],
                                     start=True, stop=True)
                    if h % 2 == 1:
                        i = h // 2
                        nc.vector.tensor_mul(ctxT[:, i, :], ctx_un[:, i, :],
                                             ps_b[:, :T])

                # --- Wo + residual + LN1 ---
                ps_mean = ps_aux.tile([P, 512], f32, name="ps_mean", tag="ax")
                ps_sq = ps_aux.tile([P, 512], f32, name="ps_sq", tag="ax")
                for m in range(0, NC, 2):
                    ps = ps_main.tile([P, 512], f32, name="ps_o", tag="mm")
                    for half in range(2):
                        for c in range(NC):
                            nc.tensor.matmul(
                                ps[:, half * T:(half + 1) * T],
                                lhsT=wo_sb[c][:, (m + half) * P:(m + half + 1) * P],
                                rhs=ctxT[:, c, :], start=(c == 0), stop=(c == NC - 1))
                    nc.vector.tensor_add(
                        z[:, m:m + 2, :].rearrange("p a t -> p (a t)"), ps[:],
                        x[:, m:m + 2, :].rearrange("p a t -> p (a t)"))
                    ln_prep_chunk(m)
                    ln_prep_chunk(m + 1)
                    if m >= 2:
                        ln_stats_chunk(ps_mean, ps_sq, m - 2)
                        ln_stats_chunk(ps_mean, ps_sq, m - 1)
                ln_stats_chunk(ps_mean, ps_sq, NC - 2)
                ln_stats_chunk(ps_mean, ps_sq, NC - 1)
                ln_tail(ps_mean, ps_sq, z, None, x1b)

                # --- FFN1 (+gelu), fine-grained weight streaming ---
                for mf in range(0, FC, 2):
                    w1_sb = [w1p.tile([P, 2 * P], bf16,
                                      name=f"w1_{l}_{mf}_{c}", tag="w1")
                             for c in range(NC)]
                    for c in range(NC):
                        nc.sync.dma_start(
                            w1_sb[c][:], w1[l, c * P:(c + 1) * P,
                                            mf * P:(mf + 2) * P])
                    ps = ps_main.tile([P, 512], f32, name="ps_f1", tag="mm")
                    for half in range(2):
                        for c in range(NC):
                            nc.tensor.matmul(
                                ps[:, half * T:(half + 1) * T],
                                lhsT=w1_sb[c][:, half * P:(half + 1) * P],
                                rhs=x1b[:, c, :],
                                start=(c == 0), stop=(c == NC - 1))
                    nc.scalar.activation(
                        hT[:, mf:mf + 2, :].rearrange("p a t -> p (a t)"),
                        ps[:], AF.Gelu, bias=zcol[:], scale=1.0)

                # next layer's K weights stream in ahead of the W2 stream
                if l + 1 < L:
                    wk_sb = load_w8(wkv, wk, l + 1, "w", "wk")

                # --- FFN2 + residual + LN2 ---
                ps_pools = {0: (ps_main, "mm", 512), 1: (ps_main, "mm", 512),
                            2: (ps_attn, "sc", T), 3: (ps_attn, "sc", T),
                            4: (ps_ctx, "cx", T), 5: (ps_ctx, "cx", T),
                            6: (ps_aux, "ax", 512), 7: (ps_aux, "ax", 512)}
                ps_acc = [ps_pools[m][0].tile([P, ps_pools[m][2]], f32,
                                              name=f"ps_f2_{m}", tag=ps_pools[m][1])
                          for m in range(NC)]
                for fc in range(FC):
                    w2_sb = w2p.tile([P, D], bf16, name="w2_sb", tag="w2")
                    nc.sync.dma_start(w2_sb[:], w2[l, fc * P:(fc + 1) * P, :])
                    for m in range(NC):
                        nc.tensor.matmul(
                            ps_acc[m][:, :T], lhsT=w2_sb[:, m * P:(m + 1) * P],
                            rhs=hT[:, fc, :], start=(fc == 0), stop=(fc == FC - 1))
                # next layer's V weights stream in behind the W2 stream
                if l + 1 < L:
                    wv_sb = load_w8(wkv, wv, l + 1, "w", "wv")
                ps_mean = ps_aux.tile([P, 512], f32, name="ps_mean", tag="ax")
                ps_sq = ps_aux.tile([P, 512], f32, name="ps_sq", tag="ax")
                for m in range(NC):
                    nc.vector.tensor_add(z[:, m, :], ps_acc[m][:, :T], x1b[:, m, :])
                    ln_prep_chunk(m)
                    if m >= 1:
                        ln_stats_chunk(ps_mean, ps_sq, m - 1)
                ln_stats_chunk(ps_mean, ps_sq, NC - 1)
                ln_tail(ps_mean, ps_sq, z, x, xb)

            nc.sync.dma_start(out[:].rearrange("c p t -> p c t"), x[:])

    nc.compile()
    return nc


_NC_CACHE = []


def get_nc():
    if not _NC_CACHE:
        _NC_CACHE.append(build_nc())
    return _NC_CACHE[0]


def prepare_in_maps(inputs):
    inp = {k: np.asarray(v) for k, v in inputs.items()}
    tokens = inp["tokens"]
    emb = inp["emb"].astype(np.float32)

    # host-side embedding lookup + positional encoding (index preprocessing)
    pe = posenc_np(S, D)
    x0 = emb[tokens] + pe[None, :, :]                     # [B, S, D] f32

    # fold the attention scale and the fp8 K-transport prescale into Wq
    # (powers of two: exact in bf16)
    wq_h = np.ascontiguousarray((inp["Wq"].astype(np.float32) * (SCALE / KVS))
                                .astype(ml_dtypes.bfloat16))
    wk_h = np.ascontiguousarray((inp["Wk"].astype(np.float32) * KVS)
                                .astype(ml_dtypes.bfloat16))
    wv_h = np.ascontiguousarray((inp["Wv"].astype(np.float32) * KVS)
                                .astype(ml_dtypes.bfloat16))
    wo_h = np.ascontiguousarray(inp["Wo"].astype(np.float32).astype(ml_dtypes.bfloat16))
    w1_h = np.ascontiguousarray(inp["W1"].astype(np.float32).astype(ml_dtypes.bfloat16))
    w2_h = np.ascontiguousarray(inp["W2"].astype(np.float32).astype(ml_dtypes.bfloat16))

    for name in ("bq", "bk", "bv", "bo"):
        assert not np.any(inp[name]), f"nonzero bias {name} not supported"
    assert np.all(inp["ln1_g"] == 1.0) and not np.any(inp["ln1_b"])
    assert np.all(inp["ln2_g"] == 1.0) and not np.any(inp["ln2_b"])

    in_maps = []
    for core in range(NCORES):
        g, r = core // NSH, core % NSH
        xs = x0[g, r * T:(r + 1) * T, :]                  # [T, D]
        x0t = np.ascontiguousarray(
            xs.T.reshape(NC, P, T).astype(np.float32))    # [NC, P, T]
        # -ln(16) bias keeps exp outputs inside fp8 range; it cancels in the
        # softmax ratio because the denominator ones-columns carry KVS=16.
        mb = np.where(tokens[g] == PAD, np.float32(-1e9),
                      np.float32(-np.log(16.0)))
        maskcol = np.ascontiguousarray(mb.reshape(NC, P).T)  # [P, NC]
        in_maps.append({
            "x0t": x0t, "maskcol": maskcol,
            "wq": wq_h, "wk": wk_h, "wv": wv_h, "wo": wo_h,
            "w1": w1_h, "w2": w2_h,
        })
    return in_maps


def assemble_output(res):
    outp = np.empty((B, S, D), np.float32)
    for core in range(NCORES):
        g, r = core // NSH, core % NSH
        o = res.results[core]["out"]                      # [NC, P, T]
        outp[g, r * T:(r + 1) * T, :] = o.reshape(D, T).T
    return outp


def kernel(**inputs):
    nc = get_nc()
    in_maps = prepare_in_maps(inputs)
    res = run_bass_kernel_spmd(nc, in_maps, core_ids=list(range(NCORES)))
    return assemble_output(res)
